# revision 7
# baseline (speedup 1.0000x reference)
"""Trainium2 Bass kernel for a post-norm transformer encoder layer (fp8).

Contract: kernel(**inputs) takes the FULL fp32 inputs (as produced by the
problem's setup_inputs) and returns the FULL [2, 2048, 512] fp32 output.

Sharding (8 cores, no collectives): core c owns 512 query tokens of batch
c // 4 (slice (c % 4) * 512). Each core recomputes the K/V projections for
its whole batch (2048 tokens) and runs attention + FFN for its 512 queries.

Speed strategy vs the bf16 baseline:
- All big matmuls run in fp8 e4m3 with MatmulPerfMode.DoubleRow (2 k-subtiles
  per matmul at 0.5 cycles/row). Weights are pre-scaled x32 on the host; the
  /32 is folded into each PSUM readout (activation scale or tensor_scalar).
- Scores are emitted pre-scaled by log2(e) (folded into the fp8 quantization
  scale of qh/kh), so softmax exp(s) == 2^scores. The exp work is split
  between the ACT engine (Exp with scale=ln2) and the Pool/GPSIMD engine
  (tensor_tensor pow with base-2 constant), which roughly doubles softmax
  throughput - the single-engine exp wall is the dominant cost otherwise.
- Scores use DoubleRow via an even/odd-partition repack of qh/kh (one plain
  reshape DMA per head: [64, N] -> [32, 2, N] pairs features (2p, 2p+1)).
- The softmax denominator rides along as a 65th output row of the AV matmul
  (ones column in vh); normalization folds the /32 of V into the reciprocal.
"""

import numpy as np
import ml_dtypes

D = 512
S = 2048
B = 2
H = 8
HD = 64
F = 2048
EPS = 1e-5
NCORES = 8
SQ = 512          # queries per core
P = 128           # partitions
KD = D // P       # 4   D-tiles
KT = S // P       # 16  key tiles
TB = S // 512     # 4   512-token blocks
FT = F // P       # 16  FFN hidden tiles

WS = 32.0                                  # host weight pre-scale
ALPHA = float(np.sqrt(np.log2(np.e) / 8))  # qh/kh scale: alpha^2 = log2e/8

F8 = ml_dtypes.float8_e4m3
BF16 = ml_dtypes.bfloat16

_CACHE = {}
LAST_RESULT = None

# aux column layout (f32 [P, 48]): per-dt vectors tiled [P, n]
AUX_BQ, AUX_BK, AUX_BO, AUX_B2 = 0, 4, 8, 12
AUX_G1, AUX_BE1, AUX_G2, AUX_BE2 = 16, 20, 24, 28
AUX_B1 = 32  # 16 cols


def _build_nc():
    import concourse.bacc as bacc
    import concourse.tile as tile
    from concourse import mybir

    bf = mybir.dt.bfloat16
    f32 = mybir.dt.float32
    f8 = mybir.dt.float8e4
    ACT = mybir.ActivationFunctionType
    DR = mybir.MatmulPerfMode.DoubleRow
    ALU = mybir.AluOpType
    LN2E = float(np.log(2.0))

    nc = bacc.Bacc("TRN2", target_bir_lowering=False, debug=False)

    def din(name, shape, dt=f8):
        return nc.dram_tensor(name, shape, dt, kind="ExternalInput").ap()

    t_aux = din("aux", [P, 48], f32)
    t_wq = din("wq8", [P, KD, D])
    t_q8 = din("qt8", [P, KD, SQ])
    t_wk = din("wk8", [P, KD, D])
    t_kt = din("kt8", [P, KD, S])
    t_wv = din("wv8", [P, KD, D])
    t_vt = din("vt8", [P, KD, S])
    t_qbf = din("qbf", [P, KD, SQ], bf)
    t_bv = din("bv32", [P, D], bf)
    t_wo = din("wo8", [HD, H, KD, P])
    t_w1 = din("w18", [P, KD, F])
    t_w2 = din("w28", [P, FT, D])
    t_out = nc.dram_tensor("outT", [P, KD, SQ], f32, kind="ExternalOutput").ap()

    with tile.TileContext(nc) as tc, \
         tc.tile_pool(name="statics", bufs=1) as SP:
        def st(shape, dt, name):
            return SP.tile(shape, dt, tag=name, name=name)

        # ---- constants (memset; no DMA) ----
        two_bf = st([P, 2, SQ], bf, "two_bf")
        nc.gpsimd.memset(two_bf, 2.0)
        ones8 = st([P, 2, 1], f8, "ones8")          # 1/D for LN stat matmuls
        nc.gpsimd.memset(ones8, 1.0 / D)
        ones_row = st([P, HD], bf, "ones_row")      # partition-bcast matmul lhsT
        nc.gpsimd.memset(ones_row, 1.0)
        one8 = st([1, 1], f8, "one8")               # warm-matmul lhsT
        nc.gpsimd.memset(one8, 1.0)
        eps_t = st([1, 1], f32, "eps_t")
        nc.gpsimd.memset(eps_t, EPS)
        warm_rhs = st([1, SQ], bf, "warm_rhs")
        nc.gpsimd.memset(warm_rhs, 0.0)
        ones_bf1 = st([1, 1], bf, "ones_bf1")
        nc.gpsimd.memset(ones_bf1, 1.0)

        # ---- input DMAs (emitted in first-use order) ----
        aux = st([P, 48], f32, "aux")
        nc.sync.dma_start(out=aux, in_=t_aux)
        wq = st([P, KD, D], f8, "wq")
        nc.sync.dma_start(out=wq, in_=t_wq)
        q8 = st([P, KD, SQ], f8, "q8")
        nc.sync.dma_start(out=q8, in_=t_q8)
        wk = st([P, KD, D], f8, "wk")
        nc.sync.dma_start(out=wk, in_=t_wk)
        kt8 = st([P, KD, S], f8, "kt8")
        nc.sync.dma_start(out=kt8[:, :, 0:1024], in_=t_kt[:, :, 0:1024])
        nc.sync.dma_start(out=kt8[:, :, 1024:2048], in_=t_kt[:, :, 1024:2048])
        wv = st([P, KD, D], f8, "wv")
        nc.sync.dma_start(out=wv, in_=t_wv)
        vt8 = st([P, KD, S], f8, "vt8")
        nc.sync.dma_start(out=vt8[:, :, 0:1024], in_=t_vt[:, :, 0:1024])
        nc.sync.dma_start(out=vt8[:, :, 1024:2048], in_=t_vt[:, :, 1024:2048])
        bv32 = st([P, D], bf, "bv32")
        nc.sync.dma_start(out=bv32, in_=t_bv)
        qbf = st([P, KD, SQ], bf, "qbf")
        nc.sync.dma_start(out=qbf, in_=t_qbf)
        wo = st([HD, H, KD, P], f8, "wo")
        nc.sync.dma_start(out=wo, in_=t_wo)
        w1 = st([P, KD, F], f8, "w1")
        nc.sync.dma_start(out=w1, in_=t_w1)
        w2 = st([P, FT, D], f8, "w2")
        nc.sync.dma_start(out=w2, in_=t_w2)

        # ---- persistent activations ----
        qh8 = st([P, KD, SQ], f8, "qh8")
        kh8 = st([P, KD, S], f8, "kh8")
        qdr = st([32, H, 2, SQ], f8, "qdr")
        kdr = st([32, H, 2, S], f8, "kdr")
        vh8 = st([P, KT, H, HD + 2], f8, "vh8")
        avt = st([P, H, SQ], f8, "avt")
        xres = st([P, KD, SQ], f32, "xres")
        x8 = st([P, KD, SQ], f8, "x8")
        sq8 = st([P, KD, SQ], f8, "sq8")
        x1f = st([P, KD, SQ], f32, "x1f")
        x1b = st([P, KD, SQ], f8, "x1b")
        hsb = st([P, FT, SQ], f8, "hsb")
        SP.seal()

        nc.gpsimd.memset(vh8[:, :, :, HD:HD + 1], 1.0)

        def aux_ap(base, i):
            return aux[:, base + i:base + i + 1]

        # round-robin PSUM->SBUF readout across the three elementwise engines
        rr_state = [0]

        def readout(out, ps, scale, bias_ap, engines=("act", "dve", "pool")):
            e = engines[rr_state[0] % len(engines)]
            rr_state[0] += 1
            if e == "act":
                nc.scalar.activation(out=out, in_=ps, func=ACT.Identity,
                                     bias=bias_ap, scale=scale)
            else:
                eng = nc.vector if e == "dve" else nc.gpsimd
                eng.tensor_scalar(out=out, in0=ps, scalar1=scale,
                                  scalar2=bias_ap, op0=ALU.mult, op1=ALU.add)

        # greedy ACT/Pool balance for the softmax exp units
        exp_t = {"act": 0.0, "pool": 0.0}
        EXP_COST = {"act": 1040.0, "pool": 1520.0}

        def exp_unit(p8, psc):
            if exp_t["act"] + EXP_COST["act"] <= exp_t["pool"] + EXP_COST["pool"]:
                e = "act"
                nc.scalar.activation(out=p8, in_=psc, func=ACT.Exp, scale=LN2E)
            else:
                e = "pool"
                nc.gpsimd.tensor_tensor(out=p8, in0=two_bf, in1=psc, op=ALU.pow)
            exp_t[e] += EXP_COST[e]

        # ---------------- phase 1: projections ----------------
        with tc.tile_pool(name="att_sb", bufs=1) as asb, \
             tc.tile_pool(name="sc", bufs=1, space="PSUM") as scp, \
             tc.tile_pool(name="av", bufs=1, space="PSUM") as avp:

            with tc.tile_pool(name="pj", bufs=2, space="PSUM") as pj:
                # keep-warm while the first DMAs land (borrows a pj buffer)
                warm_ps = pj.tile([P, SQ], f32, tag="pj", name="warm0")
                for w in range(12):
                    nc.tensor.matmul(warm_ps[0:1, :], ones_bf1, warm_rhs,
                                     start=(w == 0), stop=(w == 11))

                # Q projection (DoubleRow over k-pairs)
                for dt in range(KD):
                    ps = pj.tile([P, SQ], f32, tag="pj", name=f"pq{dt}")
                    for j in range(2):
                        nc.tensor.matmul(ps, wq[:, 2 * j:2 * j + 2,
                                                 dt * P:(dt + 1) * P],
                                         q8[:, 2 * j:2 * j + 2, :],
                                         start=(j == 0), stop=(j == 1),
                                         perf_mode=DR)
                    readout(qh8[:, dt, :], ps, ALPHA / WS, aux_ap(AUX_BQ, dt))
                    for m in range(2):  # repack [64,N] -> [32,2,N] per head
                        h = 2 * dt + m
                        nc.sync.dma_start(out=qdr[0:32, h, :, :],
                                          in_=qh8[64 * m:64 * m + 64, dt, :])

                # K projection
                for dt in range(KD):
                    for tb in range(TB):
                        tbs = slice(tb * 512, (tb + 1) * 512)
                        ps = pj.tile([P, 512], f32, tag="pj", name=f"pk{dt}_{tb}")
                        for j in range(2):
                            nc.tensor.matmul(ps, wk[:, 2 * j:2 * j + 2,
                                                     dt * P:(dt + 1) * P],
                                             kt8[:, 2 * j:2 * j + 2, tbs],
                                             start=(j == 0), stop=(j == 1),
                                             perf_mode=DR)
                        readout(kh8[:, dt, tbs], ps, ALPHA / WS,
                                aux_ap(AUX_BK, dt))
                    for m in range(2):
                        h = 2 * dt + m
                        nc.sync.dma_start(out=kdr[0:32, h, :, :],
                                          in_=kh8[64 * m:64 * m + 64, dt, :])

                # V projection: vh8 = 32*(v @ Wv^T + bv), key tokens on
                # partitions. tt 0..7 here, tt 8..15 woven into head 0.
                bv8 = bv32.rearrange("p (h d) -> p h d", h=H)

                def v_proj(tt):
                    ps = pj.tile([P, D], f32, tag="pj", name=f"pv{tt}")
                    for j in range(2):
                        nc.tensor.matmul(ps, vt8[:, 2 * j:2 * j + 2,
                                                  tt * P:(tt + 1) * P],
                                         wv[:, 2 * j:2 * j + 2, :],
                                         start=(j == 0), stop=(j == 1),
                                         perf_mode=DR)
                    eng = nc.vector if tt % 2 == 0 else nc.gpsimd
                    eng.tensor_tensor(
                        out=vh8[:, tt, :, 0:HD],
                        in0=ps.rearrange("p (h d) -> p h d", h=H),
                        in1=bv8, op=ALU.add)

                for tt in range(8):
                    v_proj(tt)

                # ---------------- phase 2: attention ----------------
                def attention_head(h, with_vproj):
                    pav = avp.tile([HD + 1, SQ], f32, tag="pav", bufs=1,
                                   name=f"pav{h}")
                    prev = None
                    for k2 in range(KT // 2):
                        psc = scp.tile([P, 2, SQ], f32, tag="psc", bufs=2)
                        for i in range(2):
                            kt = 2 * k2 + i
                            ktl = slice(kt * P, (kt + 1) * P)
                            nc.tensor.matmul(psc[:, i, :], kdr[0:32, h, :, ktl],
                                             qdr[0:32, h, :, :],
                                             start=True, stop=True, perf_mode=DR)
                        p8 = asb.tile([P, 2, SQ], f8, tag="p8", bufs=3)
                        exp_unit(p8, psc)
                        if with_vproj:
                            v_proj(8 + k2)
                        else:
                            # throttled keep-warm: anchored on the fresh p8
                            wp = wmp[0].tile([1, SQ], f32, tag="warm", bufs=1)
                            for w in range(2):
                                nc.tensor.matmul(wp, one8, p8[0:1, 0, :],
                                                 start=(w == 0), stop=(w == 1))
                        if prev is not None:
                            q0, pk2 = prev
                            nc.tensor.matmul(pav, vh8[:, 2 * pk2:2 * pk2 + 2,
                                                       h, 0:HD + 1],
                                             q0, start=(pk2 == 0), stop=False,
                                             perf_mode=DR)
                        prev = (p8, k2)
                    q0, pk2 = prev
                    nc.tensor.matmul(pav, vh8[:, 2 * pk2:2 * pk2 + 2, h, 0:HD + 1],
                                     q0, start=False, stop=True, perf_mode=DR)
                    # normalize: den is row HD of pav; fold V's x32 into rec
                    rec = asb.tile([P, SQ], f32, tag="rec", bufs=2)
                    nc.vector.reciprocal(rec[HD:HD + 1, :], pav[HD:HD + 1, :])
                    recb = asb.tile([P, SQ], bf, tag="recb", bufs=2)
                    nc.vector.tensor_scalar(out=recb[HD:HD + 1, :],
                                            in0=rec[HD:HD + 1, :],
                                            scalar1=1.0 / WS, scalar2=None,
                                            op0=ALU.mult)
                    pbc = scp.tile([HD, SQ], f32, tag="pbc", bufs=1,
                                   name=f"pbc{h}")
                    nc.tensor.matmul(pbc, ones_row[HD:HD + 1, :],
                                     recb[HD:HD + 1, :], start=True, stop=True)
                    rsb = asb.tile([HD, SQ], f32, tag="rsb", bufs=2)
                    nc.vector.tensor_copy(rsb, pbc)
                    nc.vector.tensor_tensor(out=avt[0:HD, h, :],
                                            in0=pav[0:HD, :], in1=rsb,
                                            op=ALU.mult)

                wmp = [None]
                attention_head(0, True)

            # pj pool closed: 2 banks free for the warm tile through h1..h7
            with tc.tile_pool(name="wm_att", bufs=1, space="PSUM") as wma:
                wmp[0] = wma
                for h in range(1, H):
                    attention_head(h, False)

        # ---------------- phase 3: Wo + residual ----------------
        with tc.tile_pool(name="wo_ps", bufs=1, space="PSUM") as wop:
            for dt in range(KD):
                po = wop.tile([P, SQ], f32, tag="po", bufs=2, name=f"po{dt}")
                for h in range(H):
                    nc.tensor.matmul(po, wo[0:HD, h, dt, :], avt[0:HD, h, :],
                                     start=(h == 0), stop=(h == H - 1))
                nc.scalar.activation(out=xres[:, dt, :], in_=po,
                                     func=ACT.Identity,
                                     bias=aux_ap(AUX_BO, dt), scale=1.0 / WS)
                nc.vector.tensor_tensor(out=xres[:, dt, :], in0=xres[:, dt, :],
                                        in1=qbf[:, dt, :], op=ALU.add)

        def layer_norm(src, gbase, bbase, dst_f32, dst_f8, stp, tmp):
            """dst = LN(src)*g + b over the feature (partition*KD) axis.
            Stats via fp8 DoubleRow matmuls against ones8 (=1/D)."""
            for dt in range(KD):
                eng = nc.vector if dt % 2 == 0 else nc.gpsimd
                eng.tensor_copy(x8[:, dt, :], src[:, dt, :])
                eng2 = nc.gpsimd if dt % 2 == 0 else nc.vector
                eng2.tensor_tensor(out=sq8[:, dt, :], in0=x8[:, dt, :],
                                   in1=x8[:, dt, :], op=ALU.mult)
            ps1 = stp.tile([1, SQ], f32, tag="s1")
            ps2 = stp.tile([1, SQ], f32, tag="s2")
            for jp in range(2):
                nc.tensor.matmul(ps1, ones8, x8[:, 2 * jp:2 * jp + 2, :],
                                 start=(jp == 0), stop=(jp == 1), perf_mode=DR)
            for jp in range(2):
                nc.tensor.matmul(ps2, ones8, sq8[:, 2 * jp:2 * jp + 2, :],
                                 start=(jp == 0), stop=(jp == 1), perf_mode=DR)
            mean_sb = tmp.tile([1, SQ], f32, tag="ln_mean")
            nc.vector.tensor_copy(mean_sb, ps1)
            msq = tmp.tile([1, SQ], f32, tag="ln_msq")
            nc.vector.tensor_tensor(out=msq, in0=mean_sb, in1=mean_sb,
                                    op=ALU.mult)
            var = tmp.tile([1, SQ], f32, tag="ln_var")
            nc.vector.tensor_tensor(out=var, in0=ps2, in1=msq, op=ALU.subtract)
            sd = tmp.tile([1, SQ], f32, tag="ln_sd")
            nc.scalar.activation(out=sd, in_=var, func=ACT.Sqrt, bias=eps_t)
            rstd = tmp.tile([1, SQ], f32, tag="ln_rstd")
            nc.vector.reciprocal(rstd, sd)
            cvec = tmp.tile([1, SQ], f32, tag="ln_c")
            nc.vector.tensor_tensor(out=cvec, in0=mean_sb, in1=rstd,
                                    op=ALU.mult)
            pA = tmp.tile([P, SQ], f32, tag="bA")
            nc.gpsimd.partition_broadcast(pA, rstd)
            pC = tmp.tile([P, SQ], f32, tag="bC")
            nc.gpsimd.partition_broadcast(pC, cvec)
            for dt in range(KD):
                t1 = tmp.tile([P, SQ], f32, tag="t1", bufs=2)
                nc.vector.tensor_tensor(out=t1, in0=src[:, dt, :], in1=pA,
                                        op=ALU.mult)
                nc.vector.tensor_tensor(out=t1, in0=t1, in1=pC, op=ALU.subtract)
                nc.scalar.activation(out=dst_f32[:, dt, :], in_=t1,
                                     func=ACT.Identity,
                                     bias=aux_ap(bbase, dt),
                                     scale=aux_ap(gbase, dt))
                if dst_f8 is not None:
                    nc.gpsimd.tensor_copy(dst_f8[:, dt, :], dst_f32[:, dt, :])

        with tc.tile_pool(name="ln1_sb", bufs=1) as tmp1, \
             tc.tile_pool(name="st1", bufs=1, space="PSUM") as stp1, \
             tc.tile_pool(name="wm1", bufs=1, space="PSUM") as wmp1:
            layer_norm(xres, AUX_G1, AUX_BE1, x1f, x1b, stp1, tmp1)
            # keep the PE ramp alive across the LN scalar chain
            for dt in range(KD):
                wp = wmp1.tile([1, SQ], f32, tag="wm", bufs=1)
                for w in range(4):
                    nc.tensor.matmul(wp, one8, x1b[0:1, dt, :],
                                     start=(w == 0), stop=(w == 3))

        # ---------------- phase 4: FFN ----------------
        with tc.tile_pool(name="pf", bufs=4, space="PSUM") as pfp:
            for ft in range(FT):
                pf = pfp.tile([P, SQ], f32, tag="pf")
                for j in range(2):
                    nc.tensor.matmul(pf, w1[:, 2 * j:2 * j + 2,
                                             ft * P:(ft + 1) * P],
                                     x1b[:, 2 * j:2 * j + 2, :],
                                     start=(j == 0), stop=(j == 1), perf_mode=DR)
                e = ("act", "act", "dve", "pool")[ft % 4]
                if e == "act":
                    nc.scalar.activation(out=hsb[:, ft, :], in_=pf,
                                         func=ACT.Relu,
                                         bias=aux_ap(AUX_B1, ft), scale=1.0 / WS)
                else:
                    eng = nc.vector if e == "dve" else nc.gpsimd
                    eng.tensor_scalar(out=hsb[:, ft, :], in0=pf,
                                      scalar1=1.0 / WS,
                                      scalar2=aux_ap(AUX_B1, ft),
                                      op0=ALU.mult, op1=ALU.add)
                    eng.tensor_scalar(out=hsb[:, ft, :], in0=hsb[:, ft, :],
                                      scalar1=0.0, scalar2=None, op0=ALU.max)

        r2 = xres      # dead after LN1 -> reuse for x1 + ffn
        outsb = x1f    # x1f dead per-dt after the r2 add -> reuse for LN2 out
        with tc.tile_pool(name="ln2_sb", bufs=1) as tmp2, \
             tc.tile_pool(name="py", bufs=2, space="PSUM") as pyp, \
             tc.tile_pool(name="st2", bufs=1, space="PSUM") as stp2:
            for dt in range(KD):
                py = pyp.tile([P, SQ], f32, tag="py")
                for j in range(FT // 2):
                    nc.tensor.matmul(py, w2[:, 2 * j:2 * j + 2,
                                             dt * P:(dt + 1) * P],
                                     hsb[:, 2 * j:2 * j + 2, :],
                                     start=(j == 0), stop=(j == FT // 2 - 1),
                                     perf_mode=DR)
                tr = tmp2.tile([P, SQ], f32, tag="tr", bufs=2)
                nc.scalar.activation(out=tr, in_=py, func=ACT.Identity,
                                     bias=aux_ap(AUX_B2, dt), scale=1.0 / WS)
                nc.vector.tensor_tensor(out=r2[:, dt, :], in0=tr,
                                        in1=x1f[:, dt, :], op=ALU.add)
            layer_norm(r2, AUX_G2, AUX_BE2, outsb, None, stp2, tmp2)
            nc.sync.dma_start(out=t_out, in_=outsb)

    nc.compile()
    return nc


def _get_nc():
    if "nc" not in _CACHE:
        _CACHE["nc"] = _build_nc()
    return _CACHE["nc"]


def make_in_maps(q, k, v, Wq, bq, Wk, bk, Wv, bv, Wo, bo, W1, b1, W2, b2,
                 g1, be1, g2, be2):
    f32 = np.float32

    def tile_pd(x, n):  # [n*P] -> [P, n]
        return np.asarray(x, f32).reshape(n, P).T

    def wt8(w, cols):  # torch [out, in] -> [P, in//P, out] fp8, x32
        return np.ascontiguousarray(
            (np.asarray(w, f32).T * WS).reshape(-1, P, cols).transpose(1, 0, 2)
        ).astype(F8)

    aux = np.zeros((P, 48), f32)
    aux[:, AUX_BQ:AUX_BQ + 4] = tile_pd(bq, KD) * ALPHA
    aux[:, AUX_BK:AUX_BK + 4] = tile_pd(bk, KD) * ALPHA
    aux[:, AUX_BO:AUX_BO + 4] = tile_pd(bo, KD)
    aux[:, AUX_B2:AUX_B2 + 4] = tile_pd(b2, KD)
    aux[:, AUX_G1:AUX_G1 + 4] = tile_pd(g1, KD)
    aux[:, AUX_BE1:AUX_BE1 + 4] = tile_pd(be1, KD)
    aux[:, AUX_G2:AUX_G2 + 4] = tile_pd(g2, KD)
    aux[:, AUX_BE2:AUX_BE2 + 4] = tile_pd(be2, KD)
    aux[:, AUX_B1:AUX_B1 + 16] = tile_pd(b1, FT)

    shared = {
        "aux": aux,
        "wq8": wt8(Wq, D), "wk8": wt8(Wk, D), "wv8": wt8(Wv, D),
        "w18": wt8(W1, F), "w28": wt8(W2, D),
        "wo8": np.ascontiguousarray(
            (np.asarray(Wo, f32).T * WS).reshape(H, HD, KD, P)
            .transpose(1, 0, 2, 3)).astype(F8),
        "bv32": np.ascontiguousarray(
            np.broadcast_to(np.asarray(bv, f32) * WS, (P, D))).astype(BF16),
    }

    q = np.asarray(q, f32)
    k = np.asarray(k, f32)
    v = np.asarray(v, f32)

    def fm8(x):  # [S, D] -> [P, KD, S] feature-major fp8
        return np.ascontiguousarray(
            x.T.reshape(KD, P, S).transpose(1, 0, 2)).astype(F8)

    kts = [fm8(k[b]) for b in range(B)]
    vts = [fm8(v[b]) for b in range(B)]

    in_maps = []
    for c in range(NCORES):
        b, s0 = c // 4, (c % 4) * SQ
        qt = np.ascontiguousarray(q[b, s0:s0 + SQ, :].T)          # [D, SQ]
        qt4 = np.ascontiguousarray(qt.reshape(KD, P, SQ).transpose(1, 0, 2))
        in_maps.append({
            "qt8": qt4.astype(F8), "qbf": qt4.astype(BF16),
            "kt8": kts[b], "vt8": vts[b], **shared,
        })
    return in_maps


def assemble_out(results):
    out = np.empty((B, S, D), np.float32)
    for c in range(NCORES):
        b, s0 = c // 4, (c % 4) * SQ
        # outT [P, KD, SQ]: feature dt*P+p, token t -> out[t, feature]
        out[b, s0:s0 + SQ, :] = results[c]["outT"].transpose(2, 1, 0).reshape(SQ, D)
    return out


def kernel(**inputs):
    global LAST_RESULT
    import os

    from concourse.bass_utils import run_bass_kernel_spmd

    nc = _get_nc()
    in_maps = make_in_maps(**inputs)
    try:
        res = run_bass_kernel_spmd(nc, in_maps, core_ids=list(range(NCORES)))
    except ModuleNotFoundError:
        # BASS_TRACE set but this container has no axon NTFF profile hook
        # (antenv.axon_hooks missing) — rerun untraced.
        os.environ["BASS_NEVER_TRACE"] = "1"
        res = run_bass_kernel_spmd(nc, in_maps, core_ids=list(range(NCORES)))
    LAST_RESULT = res
    return assemble_out(res.results)


# revision 16
# speedup vs baseline: 1.0057x; 1.0057x over previous
"""Trainium2 Bass kernel for a post-norm transformer encoder layer (fp8).

Contract: kernel(**inputs) takes the FULL fp32 inputs (as produced by the
problem's setup_inputs) and returns the FULL [2, 2048, 512] fp32 output.

Sharding (8 cores, no collectives): core c owns 512 query tokens of batch
c // 4 (slice (c % 4) * 512). Each core recomputes the K/V projections for
its whole batch (2048 tokens) and runs attention + FFN for its 512 queries.

Speed strategy vs the bf16 baseline:
- All big matmuls run in fp8 e4m3 with MatmulPerfMode.DoubleRow (2 k-subtiles
  per matmul at 0.5 cycles/row). Weights are pre-scaled x32 on the host; the
  /32 is folded into each PSUM readout (activation scale or tensor_scalar).
- Scores are emitted pre-scaled by log2(e) (folded into the fp8 quantization
  scale of qh/kh), so softmax exp(s) == 2^scores. Each score tile's exp is
  split in half and processed IN PARALLEL by the ACT engine (Exp, scale=ln2)
  and the Pool/GPSIMD engine (tensor_tensor pow, base-2 constant) - the
  single-engine exp wall is the dominant cost otherwise.
- Scores and Wo use DoubleRow via an even/odd-partition repack of qh/kh/avt
  (one plain reshape DMA per head: [64, N] -> [32, 2, N] pairs features
  (2p, 2p+1); score/Wo contractions are permutation-invariant).
- The softmax denominator rides along as a 65th output row of the AV matmul
  (ones column in vh); normalization folds the /32 of V into the reciprocal.
- LN broadcasts go through a PE matmul (ones x [rstd||mean*rstd]) instead of
  gpsimd partition_broadcast, keeping Pool in the `standard` ucode library
  for the whole kernel (no PseudoReloadLibraryIndex stalls).
- DMAs: inputs stream in compute order (K in halves, V after K's first half);
  repack DMAs issue from the ACT/DVE queues so they bypass the in-order SP
  queue and land as soon as their producers finish.
"""

import numpy as np
import ml_dtypes

D = 512
S = 2048
B = 2
H = 8
HD = 64
F = 2048
EPS = 1e-5
NCORES = 8
SQ = 512          # queries per core
P = 128           # partitions
KD = D // P       # 4   D-tiles
KT = S // P       # 16  key tiles
TB = S // 512     # 4   512-token blocks
FT = F // P       # 16  FFN hidden tiles

WS = 32.0                                  # host weight pre-scale
ALPHA = float(np.sqrt(np.log2(np.e) / 8))  # qh/kh scale: alpha^2 = log2e/8

F8 = ml_dtypes.float8_e4m3
BF16 = ml_dtypes.bfloat16

_CACHE = {}
LAST_RESULT = None

# aux column layout (f32 [P, 48]): per-dt vectors tiled [P, n]
AUX_BQ, AUX_BK, AUX_BO, AUX_B2 = 0, 4, 8, 12
AUX_G1, AUX_BE1, AUX_G2, AUX_BE2 = 16, 20, 24, 28
AUX_B1 = 32  # 16 cols


def _build_nc():
    import concourse.bacc as bacc
    import concourse.tile as tile
    from concourse import mybir

    bf = mybir.dt.bfloat16
    f32 = mybir.dt.float32
    f8 = mybir.dt.float8e4
    ACT = mybir.ActivationFunctionType
    DR = mybir.MatmulPerfMode.DoubleRow
    ALU = mybir.AluOpType
    LN2E = float(np.log(2.0))

    nc = bacc.Bacc("TRN2", target_bir_lowering=False, debug=False)

    def din(name, shape, dt=f8):
        return nc.dram_tensor(name, shape, dt, kind="ExternalInput").ap()

    t_aux = din("aux", [P, 48], f32)
    t_wq = din("wq8", [P, KD, D])
    t_q8 = din("qt8", [P, KD, SQ])
    t_wk = din("wk8", [P, KD, D])
    t_kt = din("kt8", [P, KD, S])
    t_wv = din("wv8", [P, KD, D])
    t_vt = din("vt8", [P, KD, S])
    t_qbf = din("qbf", [P, KD, SQ], bf)
    t_bv = din("bv32", [P, D], bf)
    t_wo = din("wodr", [32, H, 2, KD, P])
    t_w1 = din("w18", [P, KD, F])
    t_w2 = din("w28", [P, FT, D])
    t_out = nc.dram_tensor("outT", [P, KD, SQ], f32, kind="ExternalOutput").ap()

    with tile.TileContext(nc) as tc, \
         tc.tile_pool(name="statics", bufs=1) as SP:
        def st(shape, dt, name):
            return SP.tile(shape, dt, tag=name, name=name)

        # ---- constants (memset; no DMA) ----
        two_bf = st([P, SQ], bf, "two_bf")
        nc.gpsimd.memset(two_bf, 2.0)
        ones8 = st([P, 2, 1], f8, "ones8")          # 1/D for LN stat matmuls
        nc.gpsimd.memset(ones8, 1.0 / D)
        ones_row = st([P, HD], bf, "ones_row")      # den-bcast matmul lhsT
        nc.gpsimd.memset(ones_row, 1.0)
        ones_pb = st([1, P], bf, "ones_pb")         # LN-bcast matmul lhsT
        nc.gpsimd.memset(ones_pb, 1.0)
        one8 = st([1, 1], f8, "one8")               # warm-matmul lhsT
        nc.gpsimd.memset(one8, 1.0)
        eps_t = st([1, 1], f32, "eps_t")
        nc.gpsimd.memset(eps_t, EPS)
        warm_rhs = st([1, SQ], bf, "warm_rhs")
        nc.gpsimd.memset(warm_rhs, 0.0)
        ones_bf1 = st([1, 1], bf, "ones_bf1")
        nc.gpsimd.memset(ones_bf1, 1.0)

        # ---- input DMAs (SP queue, compute order) ----
        aux = st([P, 48], f32, "aux")
        nc.sync.dma_start(out=aux, in_=t_aux)
        wq = st([P, KD, D], f8, "wq")
        nc.sync.dma_start(out=wq, in_=t_wq)
        q8 = st([P, KD, SQ], f8, "q8")
        nc.sync.dma_start(out=q8, in_=t_q8)
        wk = st([P, KD, D], f8, "wk")
        nc.sync.dma_start(out=wk, in_=t_wk)
        kt8 = st([P, KD, S], f8, "kt8")
        nc.sync.dma_start(out=kt8[:, :, 0:1024], in_=t_kt[:, :, 0:1024])
        wv = st([P, KD, D], f8, "wv")
        nc.sync.dma_start(out=wv, in_=t_wv)
        vt8 = st([P, KD, S], f8, "vt8")
        nc.sync.dma_start(out=vt8[:, :, 0:1024], in_=t_vt[:, :, 0:1024])
        bv32 = st([P, D], bf, "bv32")
        nc.sync.dma_start(out=bv32, in_=t_bv)
        # late inputs (kt8/vt8 second halves, qbf, wo, w1, w2) are DMA'd
        # mid-compute so the qdr/kdr repack DMAs can jump ahead of them on
        # the in-order SP queue.
        qbf = st([P, KD, SQ], bf, "qbf")
        wo = st([32, H, 2, KD, P], f8, "wo")
        w1 = st([P, KD, F], f8, "w1")
        w2 = st([P, FT, D], f8, "w2")

        # ---- persistent activations ----
        qh8 = st([P, KD, SQ], f8, "qh8")
        kh8 = st([P, KD, S], f8, "kh8")
        qdr = st([32, H, 2, SQ], f8, "qdr")
        kdr = st([32, H, 2, S], f8, "kdr")
        vh8 = st([P, KT, H, HD + 2], f8, "vh8")
        avt = st([P, H, SQ], f8, "avt")
        avtdr = st([32, H, 2, SQ], f8, "avtdr")
        xres = st([P, KD, SQ], f32, "xres")
        x8 = st([P, KD, SQ], f8, "x8")
        sq8 = st([P, KD, SQ], f8, "sq8")
        x1f = st([P, KD, SQ], f32, "x1f")
        x1b = st([P, KD, SQ], f8, "x1b")
        hsb = st([P, FT, SQ], f8, "hsb")
        SP.seal()

        nc.gpsimd.memset(vh8[:, :, :, HD:HD + 1], 1.0)

        def aux_ap(base, i):
            return aux[:, base + i:base + i + 1]

        # round-robin PSUM->SBUF readout across the three elementwise engines
        rr_state = [0]

        def readout(out, ps, scale, bias_ap):
            e = ("act", "dve", "pool")[rr_state[0] % 3]
            rr_state[0] += 1
            if e == "act":
                nc.scalar.activation(out=out, in_=ps, func=ACT.Identity,
                                     bias=bias_ap, scale=scale)
            else:
                eng = nc.vector if e == "dve" else nc.gpsimd
                eng.tensor_scalar(out=out, in0=ps, scalar1=scale,
                                  scalar2=bias_ap, op0=ALU.mult, op1=ALU.add)

        # greedy ACT/Pool balance for the softmax exp half-tiles (512 el)
        exp_t = {"act": 0.0, "pool": 0.0}
        EXP_COST = {"act": 615.0, "pool": 810.0}

        def exp_half(p8_half, psc_half):
            if exp_t["act"] + EXP_COST["act"] <= exp_t["pool"] + EXP_COST["pool"]:
                exp_t["act"] += EXP_COST["act"]
                nc.scalar.activation(out=p8_half, in_=psc_half, func=ACT.Exp,
                                     scale=LN2E)
            else:
                exp_t["pool"] += EXP_COST["pool"]
                nc.gpsimd.tensor_tensor(out=p8_half, in0=two_bf, in1=psc_half,
                                        op=ALU.pow)

        # ---------------- phase 1: projections ----------------
        with tc.tile_pool(name="att_sb", bufs=1) as asb:

            with tc.tile_pool(name="pj", bufs=2, space="PSUM") as pj:
                # keep-warm while the first DMAs land (borrows a pj buffer)
                warm_ps = pj.tile([P, SQ], f32, tag="pj", name="warm0")
                for w in range(10):
                    nc.tensor.matmul(warm_ps[0:1, :], ones_bf1, warm_rhs,
                                     start=(w == 0), stop=(w == 9))

                # Q projection (DoubleRow over k-pairs); repack per head
                for dt in range(KD):
                    ps = pj.tile([P, SQ], f32, tag="pj", name=f"pq{dt}")
                    for j in range(2):
                        nc.tensor.matmul(ps, wq[:, 2 * j:2 * j + 2,
                                                 dt * P:(dt + 1) * P],
                                         q8[:, 2 * j:2 * j + 2, :],
                                         start=(j == 0), stop=(j == 1),
                                         perf_mode=DR)
                    readout(qh8[:, dt, :], ps, ALPHA / WS, aux_ap(AUX_BQ, dt))
                    for m in range(2):
                        h = 2 * dt + m
                        nc.sync.dma_start(out=qdr[0:32, h, :, :],
                                          in_=qh8[64 * m:64 * m + 64, dt, :])

                # K projection, tb-major so kdr halves ship early
                def k_proj(tb):
                    tbs = slice(tb * 512, (tb + 1) * 512)
                    for dt in range(KD):
                        ps = pj.tile([P, 512], f32, tag="pj", name=f"pk{dt}_{tb}")
                        for j in range(2):
                            nc.tensor.matmul(ps, wk[:, 2 * j:2 * j + 2,
                                                     dt * P:(dt + 1) * P],
                                             kt8[:, 2 * j:2 * j + 2, tbs],
                                             start=(j == 0), stop=(j == 1),
                                             perf_mode=DR)
                        readout(kh8[:, dt, tbs], ps, ALPHA / WS,
                                aux_ap(AUX_BK, dt))

                def kdr_ship(half):
                    hs = slice(half * 1024, half * 1024 + 1024)
                    for h in range(H):
                        dt, m = h // 2, h % 2
                        nc.sync.dma_start(out=kdr[0:32, h, :, hs],
                                          in_=kh8[64 * m:64 * m + 64, dt, hs])

                k_proj(0)
                k_proj(1)
                kdr_ship(0)
                nc.sync.dma_start(out=kt8[:, :, 1024:2048],
                                  in_=t_kt[:, :, 1024:2048])
                nc.sync.dma_start(out=vt8[:, :, 1024:2048],
                                  in_=t_vt[:, :, 1024:2048])

                # V projection: vh8 = 32*(v @ Wv^T + bv), key tokens on
                # partitions; ones column at HD feeds the softmax denominator.
                bv8 = bv32.rearrange("p (h d) -> p h d", h=H)

                def v_proj(tt):
                    ps = pj.tile([P, D], f32, tag="pj", name=f"pv{tt}")
                    for j in range(2):
                        nc.tensor.matmul(ps, vt8[:, 2 * j:2 * j + 2,
                                                  tt * P:(tt + 1) * P],
                                         wv[:, 2 * j:2 * j + 2, :],
                                         start=(j == 0), stop=(j == 1),
                                         perf_mode=DR)
                    eng = nc.vector if tt % 2 == 0 else nc.gpsimd
                    eng.tensor_tensor(
                        out=vh8[:, tt, :, 0:HD],
                        in0=ps.rearrange("p (h d) -> p h d", h=H),
                        in1=bv8, op=ALU.add)

                for tt in range(8):
                    v_proj(tt)
                k_proj(2)
                k_proj(3)
                kdr_ship(1)
                nc.sync.dma_start(out=qbf, in_=t_qbf)
                nc.sync.dma_start(out=wo, in_=t_wo)
                nc.sync.dma_start(out=w1, in_=t_w1)
                nc.sync.dma_start(out=w2, in_=t_w2)
                for tt in range(8, 16):
                    v_proj(tt)

            # ---------------- phase 2: attention ----------------
            att_pools = tc.tile_pool(name="sc", bufs=1, space="PSUM")
            scp = att_pools.__enter__()
            avp_cm = tc.tile_pool(name="av", bufs=1, space="PSUM")
            avp = avp_cm.__enter__()
            wmp_cm = tc.tile_pool(name="wm", bufs=1, space="PSUM")
            wmp = wmp_cm.__enter__()

            def attention_head(h):
                pav = avp.tile([HD + 1, SQ], f32, tag="pav", bufs=2,
                               name=f"pav{h}")
                prev = None
                for k2 in range(KT // 2):
                    psc = scp.tile([P, 2, SQ], f32, tag="psc", bufs=2)
                    for i in range(2):
                        kt = 2 * k2 + i
                        ktl = slice(kt * P, (kt + 1) * P)
                        nc.tensor.matmul(psc[:, i, :], kdr[0:32, h, :, ktl],
                                         qdr[0:32, h, :, :],
                                         start=True, stop=True, perf_mode=DR)
                    p8 = asb.tile([P, 2, SQ], f8, tag="p8", bufs=3)
                    exp_half(p8[:, 0, :], psc[:, 0, :])
                    exp_half(p8[:, 1, :], psc[:, 1, :])
                    # throttled keep-warm, anchored on the fresh p8
                    wp = wmp.tile([1, SQ], f32, tag="warm", bufs=1)
                    nc.tensor.matmul(wp, one8, p8[0:1, 0, :],
                                     start=True, stop=True)
                    if prev is not None:
                        q0, pk2 = prev
                        nc.tensor.matmul(pav, vh8[:, 2 * pk2:2 * pk2 + 2,
                                                   h, 0:HD + 1],
                                         q0, start=(pk2 == 0), stop=False,
                                         perf_mode=DR)
                    prev = (p8, k2)
                q0, pk2 = prev
                nc.tensor.matmul(pav, vh8[:, 2 * pk2:2 * pk2 + 2, h, 0:HD + 1],
                                 q0, start=False, stop=True, perf_mode=DR)
                # normalize: den is row HD of pav; fold V's x32 into rec
                rec = asb.tile([P, SQ], f32, tag="rec", bufs=2)
                nc.vector.reciprocal(rec[HD:HD + 1, :], pav[HD:HD + 1, :])
                recb = asb.tile([P, SQ], bf, tag="recb", bufs=2)
                nc.vector.tensor_scalar(out=recb[HD:HD + 1, :],
                                        in0=rec[HD:HD + 1, :],
                                        scalar1=1.0 / WS, scalar2=None,
                                        op0=ALU.mult)
                pbc = scp.tile([HD, SQ], f32, tag="pbc", bufs=1, name=f"pbc{h}")
                nc.tensor.matmul(pbc, ones_row[HD:HD + 1, :],
                                 recb[HD:HD + 1, :], start=True, stop=True)
                rsb = asb.tile([HD, SQ], f32, tag="rsb", bufs=2)
                nc.vector.tensor_copy(rsb, pbc)
                nc.vector.tensor_tensor(out=avt[0:HD, h, :], in0=pav[0:HD, :],
                                        in1=rsb, op=ALU.mult)
                nc.sync.dma_start(out=avtdr[0:32, h, :, :],
                                  in_=avt[0:HD, h, :])

            for h in range(H):
                attention_head(h)
            wmp_cm.__exit__(None, None, None)
            avp_cm.__exit__(None, None, None)
            att_pools.__exit__(None, None, None)

        # ---------------- phase 3: Wo (DoubleRow) + residual ----------------
        with tc.tile_pool(name="wo_ps", bufs=1, space="PSUM") as wop:
            for dt in range(KD):
                po = wop.tile([P, SQ], f32, tag="po", bufs=2, name=f"po{dt}")
                for h in range(H):
                    nc.tensor.matmul(po, wo[0:32, h, :, dt, :],
                                     avtdr[0:32, h, :, :],
                                     start=(h == 0), stop=(h == H - 1),
                                     perf_mode=DR)
                nc.scalar.activation(out=xres[:, dt, :], in_=po,
                                     func=ACT.Identity,
                                     bias=aux_ap(AUX_BO, dt), scale=1.0 / WS)
                nc.vector.tensor_tensor(out=xres[:, dt, :], in0=xres[:, dt, :],
                                        in1=qbf[:, dt, :], op=ALU.add)

        def layer_norm(src, gbase, bbase, dst_f32, dst_f8, stp, tmp):
            """dst = LN(src)*g + b over the feature (partition*KD) axis.
            Stats via fp8 DoubleRow matmuls against ones8 (=1/D); per-query
            rstd/mean broadcast via a PE matmul (no gpsimd lib switch)."""
            for dt in range(KD):
                eng = nc.vector if dt % 2 == 0 else nc.gpsimd
                eng.tensor_copy(x8[:, dt, :], src[:, dt, :])
                eng2 = nc.gpsimd if dt % 2 == 0 else nc.vector
                eng2.tensor_tensor(out=sq8[:, dt, :], in0=x8[:, dt, :],
                                   in1=x8[:, dt, :], op=ALU.mult)
            ps1 = stp.tile([1, SQ], f32, tag="s1")
            ps2 = stp.tile([1, SQ], f32, tag="s2")
            for jp in range(2):
                nc.tensor.matmul(ps1, ones8, x8[:, 2 * jp:2 * jp + 2, :],
                                 start=(jp == 0), stop=(jp == 1), perf_mode=DR)
            for jp in range(2):
                nc.tensor.matmul(ps2, ones8, sq8[:, 2 * jp:2 * jp + 2, :],
                                 start=(jp == 0), stop=(jp == 1), perf_mode=DR)
            mean_sb = tmp.tile([1, SQ], f32, tag="ln_mean")
            nc.vector.tensor_copy(mean_sb, ps1)
            msq = tmp.tile([1, SQ], f32, tag="ln_msq")
            nc.vector.tensor_tensor(out=msq, in0=mean_sb, in1=mean_sb,
                                    op=ALU.mult)
            var = tmp.tile([1, SQ], f32, tag="ln_var")
            nc.vector.tensor_tensor(out=var, in0=ps2, in1=msq, op=ALU.subtract)
            sd = tmp.tile([1, SQ], f32, tag="ln_sd")
            nc.scalar.activation(out=sd, in_=var, func=ACT.Sqrt, bias=eps_t)
            rstd = tmp.tile([1, SQ], f32, tag="ln_rstd")
            nc.vector.reciprocal(rstd, sd)
            # ACbf = [rstd || mean*rstd] in bf16; broadcast with one matmul
            acbf = tmp.tile([1, 2, SQ], bf, tag="ln_ac")
            nc.vector.tensor_copy(acbf[:, 0, :], rstd)
            nc.vector.tensor_tensor(out=acbf[:, 1, :], in0=mean_sb, in1=rstd,
                                    op=ALU.mult)
            pac = stp.tile([P, 2, SQ], f32, tag="pac")
            nc.tensor.matmul(pac, ones_pb, acbf, start=True, stop=True)
            for dt in range(KD):
                t1 = tmp.tile([P, SQ], f32, tag="t1", bufs=2)
                nc.vector.tensor_tensor(out=t1, in0=src[:, dt, :],
                                        in1=pac[:, 0, :], op=ALU.mult)
                nc.vector.tensor_tensor(out=t1, in0=t1, in1=pac[:, 1, :],
                                        op=ALU.subtract)
                nc.scalar.activation(out=dst_f32[:, dt, :], in_=t1,
                                     func=ACT.Identity,
                                     bias=aux_ap(bbase, dt),
                                     scale=aux_ap(gbase, dt))
                if dst_f8 is not None:
                    nc.gpsimd.tensor_copy(dst_f8[:, dt, :], dst_f32[:, dt, :])

        with tc.tile_pool(name="ln1_sb", bufs=1) as tmp1, \
             tc.tile_pool(name="st1", bufs=1, space="PSUM") as stp1, \
             tc.tile_pool(name="wm1", bufs=1, space="PSUM") as wmp1:
            layer_norm(xres, AUX_G1, AUX_BE1, x1f, x1b, stp1, tmp1)
            # keep the PE ramp alive across the LN scalar chain
            for dt in range(KD):
                wp = wmp1.tile([1, SQ], f32, tag="wm", bufs=1)
                for w in range(4):
                    nc.tensor.matmul(wp, one8, x1b[0:1, dt, :],
                                     start=(w == 0), stop=(w == 3))

        # ---------------- phase 4: FFN ----------------
        with tc.tile_pool(name="pf", bufs=4, space="PSUM") as pfp:
            for ft in range(FT):
                pf = pfp.tile([P, SQ], f32, tag="pf")
                for j in range(2):
                    nc.tensor.matmul(pf, w1[:, 2 * j:2 * j + 2,
                                             ft * P:(ft + 1) * P],
                                     x1b[:, 2 * j:2 * j + 2, :],
                                     start=(j == 0), stop=(j == 1), perf_mode=DR)
                e = ("act", "act", "dve", "pool")[ft % 4]
                if e == "act":
                    nc.scalar.activation(out=hsb[:, ft, :], in_=pf,
                                         func=ACT.Relu,
                                         bias=aux_ap(AUX_B1, ft), scale=1.0 / WS)
                else:
                    eng = nc.vector if e == "dve" else nc.gpsimd
                    eng.tensor_scalar(out=hsb[:, ft, :], in0=pf,
                                      scalar1=1.0 / WS,
                                      scalar2=aux_ap(AUX_B1, ft),
                                      op0=ALU.mult, op1=ALU.add)
                    eng.tensor_scalar(out=hsb[:, ft, :], in0=hsb[:, ft, :],
                                      scalar1=0.0, scalar2=None, op0=ALU.max)

        r2 = xres      # dead after LN1 -> reuse for x1 + ffn
        outsb = x1f    # x1f dead per-dt after the r2 add -> reuse for LN2 out
        with tc.tile_pool(name="ln2_sb", bufs=1) as tmp2, \
             tc.tile_pool(name="py", bufs=2, space="PSUM") as pyp, \
             tc.tile_pool(name="st2", bufs=1, space="PSUM") as stp2:
            for dt in range(KD):
                py = pyp.tile([P, SQ], f32, tag="py")
                for j in range(FT // 2):
                    nc.tensor.matmul(py, w2[:, 2 * j:2 * j + 2,
                                             dt * P:(dt + 1) * P],
                                     hsb[:, 2 * j:2 * j + 2, :],
                                     start=(j == 0), stop=(j == FT // 2 - 1),
                                     perf_mode=DR)
                tr = tmp2.tile([P, SQ], f32, tag="tr", bufs=2)
                nc.scalar.activation(out=tr, in_=py, func=ACT.Identity,
                                     bias=aux_ap(AUX_B2, dt), scale=1.0 / WS)
                nc.vector.tensor_tensor(out=r2[:, dt, :], in0=tr,
                                        in1=x1f[:, dt, :], op=ALU.add)
            layer_norm(r2, AUX_G2, AUX_BE2, outsb, None, stp2, tmp2)
            nc.sync.dma_start(out=t_out, in_=outsb)

    nc.compile()
    return nc


def _get_nc():
    if "nc" not in _CACHE:
        _CACHE["nc"] = _build_nc()
    return _CACHE["nc"]


def make_in_maps(q, k, v, Wq, bq, Wk, bk, Wv, bv, Wo, bo, W1, b1, W2, b2,
                 g1, be1, g2, be2):
    f32 = np.float32

    def tile_pd(x, n):  # [n*P] -> [P, n]
        return np.asarray(x, f32).reshape(n, P).T

    def wt8(w, cols):  # torch [out, in] -> [P, in//P, out] fp8, x32
        return np.ascontiguousarray(
            (np.asarray(w, f32).T * WS).reshape(-1, P, cols).transpose(1, 0, 2)
        ).astype(F8)

    aux = np.zeros((P, 48), f32)
    aux[:, AUX_BQ:AUX_BQ + 4] = tile_pd(bq, KD) * ALPHA
    aux[:, AUX_BK:AUX_BK + 4] = tile_pd(bk, KD) * ALPHA
    aux[:, AUX_BO:AUX_BO + 4] = tile_pd(bo, KD)
    aux[:, AUX_B2:AUX_B2 + 4] = tile_pd(b2, KD)
    aux[:, AUX_G1:AUX_G1 + 4] = tile_pd(g1, KD)
    aux[:, AUX_BE1:AUX_BE1 + 4] = tile_pd(be1, KD)
    aux[:, AUX_G2:AUX_G2 + 4] = tile_pd(g2, KD)
    aux[:, AUX_BE2:AUX_BE2 + 4] = tile_pd(be2, KD)
    aux[:, AUX_B1:AUX_B1 + 16] = tile_pd(b1, FT)

    # Wo for DoubleRow: [32, H, 2, KD, P]; within-head feature d = 2p+j
    wodr = np.ascontiguousarray(
        (np.asarray(Wo, f32).T * WS).reshape(H, 32, 2, KD, P)
        .transpose(1, 0, 2, 3, 4)).astype(F8)

    shared = {
        "aux": aux,
        "wq8": wt8(Wq, D), "wk8": wt8(Wk, D), "wv8": wt8(Wv, D),
        "w18": wt8(W1, F), "w28": wt8(W2, D),
        "wodr": wodr,
        "bv32": np.ascontiguousarray(
            np.broadcast_to(np.asarray(bv, f32) * WS, (P, D))).astype(BF16),
    }

    q = np.asarray(q, f32)
    k = np.asarray(k, f32)
    v = np.asarray(v, f32)

    def fm8(x):  # [S, D] -> [P, KD, S] feature-major fp8
        return np.ascontiguousarray(
            x.T.reshape(KD, P, S).transpose(1, 0, 2)).astype(F8)

    kts = [fm8(k[b]) for b in range(B)]
    vts = [fm8(v[b]) for b in range(B)]

    in_maps = []
    for c in range(NCORES):
        b, s0 = c // 4, (c % 4) * SQ
        qt = np.ascontiguousarray(q[b, s0:s0 + SQ, :].T)          # [D, SQ]
        qt4 = np.ascontiguousarray(qt.reshape(KD, P, SQ).transpose(1, 0, 2))
        in_maps.append({
            "qt8": qt4.astype(F8), "qbf": qt4.astype(BF16),
            "kt8": kts[b], "vt8": vts[b], **shared,
        })
    return in_maps


def assemble_out(results):
    out = np.empty((B, S, D), np.float32)
    for c in range(NCORES):
        b, s0 = c // 4, (c % 4) * SQ
        # outT [P, KD, SQ]: feature dt*P+p, token t -> out[t, feature]
        out[b, s0:s0 + SQ, :] = results[c]["outT"].transpose(2, 1, 0).reshape(SQ, D)
    return out


def kernel(**inputs):
    global LAST_RESULT
    import os

    from concourse.bass_utils import run_bass_kernel_spmd

    nc = _get_nc()
    in_maps = make_in_maps(**inputs)
    try:
        res = run_bass_kernel_spmd(nc, in_maps, core_ids=list(range(NCORES)))
    except ModuleNotFoundError:
        # BASS_TRACE set but this container has no axon NTFF profile hook
        # (antenv.axon_hooks missing) — rerun untraced.
        os.environ["BASS_NEVER_TRACE"] = "1"
        res = run_bass_kernel_spmd(nc, in_maps, core_ids=list(range(NCORES)))
    LAST_RESULT = res
    return assemble_out(res.results)


# revision 17
# speedup vs baseline: 1.0078x; 1.0021x over previous
"""Trainium2 Bass kernel for a post-norm transformer encoder layer (fp8).

Contract: kernel(**inputs) takes the FULL fp32 inputs (as produced by the
problem's setup_inputs) and returns the FULL [2, 2048, 512] fp32 output.

Sharding (8 cores, no collectives): core c owns 512 query tokens of batch
c // 4 (slice (c % 4) * 512). Each core recomputes the K/V projections for
its whole batch (2048 tokens) and runs attention + FFN for its 512 queries.

Speed strategy vs the bf16 baseline:
- All big matmuls run in fp8 e4m3 with MatmulPerfMode.DoubleRow (2 k-subtiles
  per matmul at 0.5 cycles/row). Weights are pre-scaled x32 on the host; the
  /32 is folded into each PSUM readout (activation scale or tensor_scalar).
- Scores are emitted pre-scaled by log2(e) (folded into the fp8 quantization
  scale of qh/kh), so softmax exp(s) == 2^scores. Each score tile's exp is
  split in half and processed IN PARALLEL by the ACT engine (Exp, scale=ln2)
  and the Pool/GPSIMD engine (tensor_tensor pow, base-2 constant) - the
  single-engine exp wall is the dominant cost otherwise.
- Scores and Wo use DoubleRow via an even/odd-partition repack of qh/kh/avt
  (one plain reshape DMA per head: [64, N] -> [32, 2, N] pairs features
  (2p, 2p+1); score/Wo contractions are permutation-invariant).
- The softmax denominator rides along as a 65th output row of the AV matmul
  (ones column in vh); normalization folds the /32 of V into the reciprocal.
- LN broadcasts go through a PE matmul (ones x [rstd||mean*rstd]) instead of
  gpsimd partition_broadcast, keeping Pool in the `standard` ucode library
  for the whole kernel (no PseudoReloadLibraryIndex stalls).
- DMAs: inputs stream in compute order (K in halves, V after K's first half);
  repack DMAs issue from the ACT/DVE queues so they bypass the in-order SP
  queue and land as soon as their producers finish.
"""

import numpy as np
import ml_dtypes

D = 512
S = 2048
B = 2
H = 8
HD = 64
F = 2048
EPS = 1e-5
NCORES = 8
SQ = 512          # queries per core
P = 128           # partitions
KD = D // P       # 4   D-tiles
KT = S // P       # 16  key tiles
TB = S // 512     # 4   512-token blocks
FT = F // P       # 16  FFN hidden tiles

WS = 32.0                                  # host weight pre-scale
ALPHA = float(np.sqrt(np.log2(np.e) / 8))  # qh/kh scale: alpha^2 = log2e/8

F8 = ml_dtypes.float8_e4m3
BF16 = ml_dtypes.bfloat16

_CACHE = {}
LAST_RESULT = None

# aux column layout (f32 [P, 48]): per-dt vectors tiled [P, n]
AUX_BQ, AUX_BK, AUX_BO, AUX_B2 = 0, 4, 8, 12
AUX_G1, AUX_BE1, AUX_G2, AUX_BE2 = 16, 20, 24, 28
AUX_B1 = 32  # 16 cols


def _build_nc():
    import concourse.bacc as bacc
    import concourse.tile as tile
    from concourse import mybir

    bf = mybir.dt.bfloat16
    f32 = mybir.dt.float32
    f8 = mybir.dt.float8e4
    ACT = mybir.ActivationFunctionType
    DR = mybir.MatmulPerfMode.DoubleRow
    ALU = mybir.AluOpType
    LN2E = float(np.log(2.0))

    nc = bacc.Bacc("TRN2", target_bir_lowering=False, debug=False)

    def din(name, shape, dt=f8):
        return nc.dram_tensor(name, shape, dt, kind="ExternalInput").ap()

    t_aux = din("aux", [P, 48], f32)
    t_wq = din("wq8", [P, KD, D])
    t_q8 = din("qt8", [P, KD, SQ])
    t_wk = din("wk8", [P, KD, D])
    t_kt = din("kt8", [P, KD, S])
    t_wv = din("wv8", [P, KD, D])
    t_vt = din("vt8", [P, KD, S])
    t_qbf = din("qbf", [P, KD, SQ], bf)
    t_bv = din("bv32", [P, D], bf)
    t_wo = din("wodr", [32, H, 2, KD, P])
    t_w1 = din("w18", [P, KD, F])
    t_w2 = din("w28", [P, FT, D])
    t_out = nc.dram_tensor("outT", [P, KD, SQ], f32, kind="ExternalOutput").ap()

    with tile.TileContext(nc) as tc, \
         tc.tile_pool(name="statics", bufs=1) as SP:
        def st(shape, dt, name):
            return SP.tile(shape, dt, tag=name, name=name)

        # ---- constants (memset; no DMA) ----
        two_bf = st([P, SQ], bf, "two_bf")
        nc.gpsimd.memset(two_bf, 2.0)
        ones8 = st([P, 2, 1], f8, "ones8")          # 1/D for LN stat matmuls
        nc.gpsimd.memset(ones8, 1.0 / D)
        ones_row = st([P, HD], bf, "ones_row")      # den-bcast matmul lhsT
        nc.gpsimd.memset(ones_row, 1.0)
        ones_pb = st([1, P], bf, "ones_pb")         # LN-bcast matmul lhsT
        nc.gpsimd.memset(ones_pb, 1.0)
        one8 = st([1, 1], f8, "one8")               # warm-matmul lhsT
        nc.gpsimd.memset(one8, 1.0)
        eps_t = st([1, 1], f32, "eps_t")
        nc.gpsimd.memset(eps_t, EPS)
        warm_rhs = st([1, SQ], bf, "warm_rhs")
        nc.gpsimd.memset(warm_rhs, 0.0)
        ones_bf1 = st([1, 1], bf, "ones_bf1")
        nc.gpsimd.memset(ones_bf1, 1.0)

        # ---- input DMAs (SP queue, compute order) ----
        aux = st([P, 48], f32, "aux")
        nc.sync.dma_start(out=aux, in_=t_aux)
        wq = st([P, KD, D], f8, "wq")
        nc.sync.dma_start(out=wq, in_=t_wq)
        q8 = st([P, KD, SQ], f8, "q8")
        nc.sync.dma_start(out=q8, in_=t_q8)
        wk = st([P, KD, D], f8, "wk")
        nc.sync.dma_start(out=wk, in_=t_wk)
        kt8 = st([P, KD, S], f8, "kt8")
        nc.sync.dma_start(out=kt8[:, :, 0:1024], in_=t_kt[:, :, 0:1024])
        wv = st([P, KD, D], f8, "wv")
        nc.sync.dma_start(out=wv, in_=t_wv)
        vt8 = st([P, KD, S], f8, "vt8")
        nc.sync.dma_start(out=vt8[:, :, 0:1024], in_=t_vt[:, :, 0:1024])
        bv32 = st([P, D], bf, "bv32")
        nc.sync.dma_start(out=bv32, in_=t_bv)
        # late inputs (kt8/vt8 second halves, qbf, wo, w1, w2) are DMA'd
        # mid-compute so the qdr/kdr repack DMAs can jump ahead of them on
        # the in-order SP queue.
        qbf = st([P, KD, SQ], bf, "qbf")
        wo = st([32, H, 2, KD, P], f8, "wo")
        w1 = st([P, KD, F], f8, "w1")
        w2 = st([P, FT, D], f8, "w2")

        # ---- persistent activations ----
        qh8 = st([P, KD, SQ], f8, "qh8")
        kh8 = st([P, KD, S], f8, "kh8")
        qdr = st([32, H, 2, SQ], f8, "qdr")
        kdr = st([32, H, 2, S], f8, "kdr")
        vh8 = st([P, KT, H, HD + 2], f8, "vh8")
        avt = st([P, H, SQ], f8, "avt")
        avtdr = st([32, H, 2, SQ], f8, "avtdr")
        xres = st([P, KD, SQ], f32, "xres")
        x8 = st([P, KD, SQ], f8, "x8")
        sq8 = st([P, KD, SQ], f8, "sq8")
        x1f = st([P, KD, SQ], f32, "x1f")
        x1b = st([P, KD, SQ], f8, "x1b")
        hsb = st([P, FT, SQ], f8, "hsb")
        SP.seal()

        nc.gpsimd.memset(vh8[:, :, :, HD:HD + 1], 1.0)

        def aux_ap(base, i):
            return aux[:, base + i:base + i + 1]

        # round-robin PSUM->SBUF readout across the three elementwise engines
        rr_state = [0]

        def readout(out, ps, scale, bias_ap):
            e = ("act", "dve", "pool")[rr_state[0] % 3]
            rr_state[0] += 1
            if e == "act":
                nc.scalar.activation(out=out, in_=ps, func=ACT.Identity,
                                     bias=bias_ap, scale=scale)
            else:
                eng = nc.vector if e == "dve" else nc.gpsimd
                eng.tensor_scalar(out=out, in0=ps, scalar1=scale,
                                  scalar2=bias_ap, op0=ALU.mult, op1=ALU.add)

        # greedy ACT/Pool balance for the softmax exp half-tiles (512 el)
        exp_t = {"act": 0.0, "pool": 0.0}
        EXP_COST = {"act": 615.0, "pool": 810.0}

        def exp_half(p8_half, psc_half):
            if exp_t["act"] + EXP_COST["act"] <= exp_t["pool"] + EXP_COST["pool"]:
                exp_t["act"] += EXP_COST["act"]
                nc.scalar.activation(out=p8_half, in_=psc_half, func=ACT.Exp,
                                     scale=LN2E)
            else:
                exp_t["pool"] += EXP_COST["pool"]
                nc.gpsimd.tensor_tensor(out=p8_half, in0=two_bf, in1=psc_half,
                                        op=ALU.pow)

        # ---------------- phase 1: projections ----------------
        with tc.tile_pool(name="att_sb", bufs=1) as asb:

            with tc.tile_pool(name="pj", bufs=2, space="PSUM") as pj:
                # keep-warm while the first DMAs land (borrows a pj buffer)
                warm_ps = pj.tile([P, SQ], f32, tag="pj", name="warm0")
                for w in range(10):
                    nc.tensor.matmul(warm_ps[0:1, :], ones_bf1, warm_rhs,
                                     start=(w == 0), stop=(w == 9))

                # Q projection (DoubleRow over k-pairs); repack per head
                for dt in range(KD):
                    ps = pj.tile([P, SQ], f32, tag="pj", name=f"pq{dt}")
                    for j in range(2):
                        nc.tensor.matmul(ps, wq[:, 2 * j:2 * j + 2,
                                                 dt * P:(dt + 1) * P],
                                         q8[:, 2 * j:2 * j + 2, :],
                                         start=(j == 0), stop=(j == 1),
                                         perf_mode=DR)
                    readout(qh8[:, dt, :], ps, ALPHA / WS, aux_ap(AUX_BQ, dt))
                    for m in range(2):
                        h = 2 * dt + m
                        nc.sync.dma_start(out=qdr[0:32, h, :, :],
                                          in_=qh8[64 * m:64 * m + 64, dt, :])

                # K projection, tb-major so kdr halves ship early
                def k_proj(tb):
                    tbs = slice(tb * 512, (tb + 1) * 512)
                    for dt in range(KD):
                        ps = pj.tile([P, 512], f32, tag="pj", name=f"pk{dt}_{tb}")
                        for j in range(2):
                            nc.tensor.matmul(ps, wk[:, 2 * j:2 * j + 2,
                                                     dt * P:(dt + 1) * P],
                                             kt8[:, 2 * j:2 * j + 2, tbs],
                                             start=(j == 0), stop=(j == 1),
                                             perf_mode=DR)
                        readout(kh8[:, dt, tbs], ps, ALPHA / WS,
                                aux_ap(AUX_BK, dt))

                def kdr_ship(half):
                    hs = slice(half * 1024, half * 1024 + 1024)
                    for h in range(H):
                        dt, m = h // 2, h % 2
                        nc.sync.dma_start(out=kdr[0:32, h, :, hs],
                                          in_=kh8[64 * m:64 * m + 64, dt, hs])

                k_proj(0)
                k_proj(1)
                kdr_ship(0)
                nc.sync.dma_start(out=kt8[:, :, 1024:2048],
                                  in_=t_kt[:, :, 1024:2048])
                nc.sync.dma_start(out=vt8[:, :, 1024:2048],
                                  in_=t_vt[:, :, 1024:2048])

                # V projection: vh8 = 32*(v @ Wv^T + bv), key tokens on
                # partitions; ones column at HD feeds the softmax denominator.
                bv8 = bv32.rearrange("p (h d) -> p h d", h=H)

                def v_proj(tt):
                    ps = pj.tile([P, D], f32, tag="pj", name=f"pv{tt}")
                    for j in range(2):
                        nc.tensor.matmul(ps, vt8[:, 2 * j:2 * j + 2,
                                                  tt * P:(tt + 1) * P],
                                         wv[:, 2 * j:2 * j + 2, :],
                                         start=(j == 0), stop=(j == 1),
                                         perf_mode=DR)
                    eng = nc.vector if tt % 2 == 0 else nc.gpsimd
                    eng.tensor_tensor(
                        out=vh8[:, tt, :, 0:HD],
                        in0=ps.rearrange("p (h d) -> p h d", h=H),
                        in1=bv8, op=ALU.add)

                for tt in range(8):
                    v_proj(tt)
                k_proj(2)
                k_proj(3)
                kdr_ship(1)
                nc.sync.dma_start(out=qbf, in_=t_qbf)
                nc.sync.dma_start(out=wo, in_=t_wo)
                nc.sync.dma_start(out=w1, in_=t_w1)
                nc.sync.dma_start(out=w2, in_=t_w2)
                for tt in range(8, 16):
                    v_proj(tt)

            # ---------------- phase 2: attention ----------------
            att_pools = tc.tile_pool(name="sc", bufs=1, space="PSUM")
            scp = att_pools.__enter__()
            avp_cm = tc.tile_pool(name="av", bufs=1, space="PSUM")
            avp = avp_cm.__enter__()
            wmp_cm = tc.tile_pool(name="wm", bufs=1, space="PSUM")
            wmp = wmp_cm.__enter__()

            def attention_head(h):
                pav = avp.tile([HD + 1, SQ], f32, tag="pav", bufs=2,
                               name=f"pav{h}")
                prev = None
                for k2 in range(KT // 2):
                    psc = scp.tile([P, 2, SQ], f32, tag="psc", bufs=2)
                    for i in range(2):
                        kt = 2 * k2 + i
                        ktl = slice(kt * P, (kt + 1) * P)
                        nc.tensor.matmul(psc[:, i, :], kdr[0:32, h, :, ktl],
                                         qdr[0:32, h, :, :],
                                         start=True, stop=True, perf_mode=DR)
                    p8 = asb.tile([P, 2, SQ], f8, tag="p8", bufs=3)
                    exp_half(p8[:, 0, :], psc[:, 0, :])
                    exp_half(p8[:, 1, :], psc[:, 1, :])
                    # throttled keep-warm, anchored on the fresh p8
                    wp = wmp.tile([1, SQ], f32, tag="warm", bufs=1)
                    nc.tensor.matmul(wp, one8, p8[0:1, 0, :],
                                     start=True, stop=True)
                    if prev is not None:
                        q0, pk2 = prev
                        nc.tensor.matmul(pav, vh8[:, 2 * pk2:2 * pk2 + 2,
                                                   h, 0:HD + 1],
                                         q0, start=(pk2 == 0), stop=False,
                                         perf_mode=DR)
                    prev = (p8, k2)
                q0, pk2 = prev
                nc.tensor.matmul(pav, vh8[:, 2 * pk2:2 * pk2 + 2, h, 0:HD + 1],
                                 q0, start=False, stop=True, perf_mode=DR)
                # normalize: den is row HD of pav; fold V's x32 into rec
                rec = asb.tile([P, SQ], f32, tag="rec", bufs=2)
                nc.vector.reciprocal(rec[HD:HD + 1, :], pav[HD:HD + 1, :])
                recb = asb.tile([P, SQ], bf, tag="recb", bufs=2)
                nc.vector.tensor_scalar(out=recb[HD:HD + 1, :],
                                        in0=rec[HD:HD + 1, :],
                                        scalar1=1.0 / WS, scalar2=None,
                                        op0=ALU.mult)
                pbc = scp.tile([HD, SQ], f32, tag="pbc", bufs=1, name=f"pbc{h}")
                nc.tensor.matmul(pbc, ones_row[HD:HD + 1, :],
                                 recb[HD:HD + 1, :], start=True, stop=True)
                rsb = asb.tile([HD, SQ], f32, tag="rsb", bufs=2)
                nc.vector.tensor_copy(rsb, pbc)
                nc.vector.tensor_tensor(out=avt[0:HD, h, :], in0=pav[0:HD, :],
                                        in1=rsb, op=ALU.mult)
                nc.sync.dma_start(out=avtdr[0:32, h, :, :],
                                  in_=avt[0:HD, h, :])

            for h in range(H):
                attention_head(h)
            wmp_cm.__exit__(None, None, None)
            avp_cm.__exit__(None, None, None)
            att_pools.__exit__(None, None, None)

        # ---------------- phase 3: Wo (DoubleRow) + residual ----------------
        with tc.tile_pool(name="wo_ps", bufs=1, space="PSUM") as wop:
            for dt in range(KD):
                po = wop.tile([P, SQ], f32, tag="po", bufs=2, name=f"po{dt}")
                for h in range(H):
                    nc.tensor.matmul(po, wo[0:32, h, :, dt, :],
                                     avtdr[0:32, h, :, :],
                                     start=(h == 0), stop=(h == H - 1),
                                     perf_mode=DR)
                nc.scalar.activation(out=xres[:, dt, :], in_=po,
                                     func=ACT.Identity,
                                     bias=aux_ap(AUX_BO, dt), scale=1.0 / WS)
                nc.vector.tensor_tensor(out=xres[:, dt, :], in0=xres[:, dt, :],
                                        in1=qbf[:, dt, :], op=ALU.add)

        def layer_norm(src, gbase, bbase, dst_f32, dst_f8, stp, tmp):
            """dst = LN(src)*g + b over the feature (partition*KD) axis.
            Stats via fp8 DoubleRow matmuls against ones8 (=1/D); per-query
            rstd/mean broadcast via a PE matmul (no gpsimd lib switch)."""
            for dt in range(KD):
                eng = nc.vector if dt % 2 == 0 else nc.gpsimd
                eng.tensor_copy(x8[:, dt, :], src[:, dt, :])
                eng2 = nc.gpsimd if dt % 2 == 0 else nc.vector
                eng2.tensor_tensor(out=sq8[:, dt, :], in0=x8[:, dt, :],
                                   in1=x8[:, dt, :], op=ALU.mult)
            ps1 = stp.tile([1, SQ], f32, tag="s1")
            ps2 = stp.tile([1, SQ], f32, tag="s2")
            for jp in range(2):
                nc.tensor.matmul(ps1, ones8, x8[:, 2 * jp:2 * jp + 2, :],
                                 start=(jp == 0), stop=(jp == 1), perf_mode=DR)
            for jp in range(2):
                nc.tensor.matmul(ps2, ones8, sq8[:, 2 * jp:2 * jp + 2, :],
                                 start=(jp == 0), stop=(jp == 1), perf_mode=DR)
            mean_sb = tmp.tile([1, SQ], f32, tag="ln_mean")
            nc.vector.tensor_copy(mean_sb, ps1)
            msq = tmp.tile([1, SQ], f32, tag="ln_msq")
            nc.vector.tensor_tensor(out=msq, in0=mean_sb, in1=mean_sb,
                                    op=ALU.mult)
            var = tmp.tile([1, SQ], f32, tag="ln_var")
            nc.vector.tensor_tensor(out=var, in0=ps2, in1=msq, op=ALU.subtract)
            sd = tmp.tile([1, SQ], f32, tag="ln_sd")
            nc.scalar.activation(out=sd, in_=var, func=ACT.Sqrt, bias=eps_t)
            rstd = tmp.tile([1, SQ], f32, tag="ln_rstd")
            nc.vector.reciprocal(rstd, sd)
            # ACbf = [rstd || mean*rstd] in bf16; broadcast with one matmul
            acbf = tmp.tile([1, 2, SQ], bf, tag="ln_ac")
            nc.vector.tensor_copy(acbf[:, 0, :], rstd)
            nc.vector.tensor_tensor(out=acbf[:, 1, :], in0=mean_sb, in1=rstd,
                                    op=ALU.mult)
            pac = stp.tile([P, 2, SQ], f32, tag="pac")
            nc.tensor.matmul(pac[:, 0, :], ones_pb, acbf[:, 0, :],
                             start=True, stop=True)
            nc.tensor.matmul(pac[:, 1, :], ones_pb, acbf[:, 1, :],
                             start=True, stop=True)
            for dt in range(KD):
                t1 = tmp.tile([P, SQ], f32, tag="t1", bufs=2)
                nc.vector.tensor_tensor(out=t1, in0=src[:, dt, :],
                                        in1=pac[:, 0, :], op=ALU.mult)
                nc.vector.tensor_tensor(out=t1, in0=t1, in1=pac[:, 1, :],
                                        op=ALU.subtract)
                nc.scalar.activation(out=dst_f32[:, dt, :], in_=t1,
                                     func=ACT.Identity,
                                     bias=aux_ap(bbase, dt),
                                     scale=aux_ap(gbase, dt))
                if dst_f8 is not None:
                    nc.gpsimd.tensor_copy(dst_f8[:, dt, :], dst_f32[:, dt, :])

        with tc.tile_pool(name="ln1_sb", bufs=1) as tmp1, \
             tc.tile_pool(name="st1", bufs=1, space="PSUM") as stp1, \
             tc.tile_pool(name="wm1", bufs=1, space="PSUM") as wmp1:
            layer_norm(xres, AUX_G1, AUX_BE1, x1f, x1b, stp1, tmp1)
            # keep the PE ramp alive across the LN scalar chain
            for dt in range(KD):
                wp = wmp1.tile([1, SQ], f32, tag="wm", bufs=1)
                for w in range(4):
                    nc.tensor.matmul(wp, one8, x1b[0:1, dt, :],
                                     start=(w == 0), stop=(w == 3))

        # ---------------- phase 4: FFN ----------------
        with tc.tile_pool(name="pf", bufs=4, space="PSUM") as pfp:
            for ft in range(FT):
                pf = pfp.tile([P, SQ], f32, tag="pf")
                for j in range(2):
                    nc.tensor.matmul(pf, w1[:, 2 * j:2 * j + 2,
                                             ft * P:(ft + 1) * P],
                                     x1b[:, 2 * j:2 * j + 2, :],
                                     start=(j == 0), stop=(j == 1), perf_mode=DR)
                e = ("act", "act", "dve", "pool")[ft % 4]
                if e == "act":
                    nc.scalar.activation(out=hsb[:, ft, :], in_=pf,
                                         func=ACT.Relu,
                                         bias=aux_ap(AUX_B1, ft), scale=1.0 / WS)
                else:
                    eng = nc.vector if e == "dve" else nc.gpsimd
                    eng.tensor_scalar(out=hsb[:, ft, :], in0=pf,
                                      scalar1=1.0 / WS,
                                      scalar2=aux_ap(AUX_B1, ft),
                                      op0=ALU.mult, op1=ALU.add)
                    eng.tensor_scalar(out=hsb[:, ft, :], in0=hsb[:, ft, :],
                                      scalar1=0.0, scalar2=None, op0=ALU.max)

        r2 = xres      # dead after LN1 -> reuse for x1 + ffn
        outsb = x1f    # x1f dead per-dt after the r2 add -> reuse for LN2 out
        with tc.tile_pool(name="ln2_sb", bufs=1) as tmp2, \
             tc.tile_pool(name="py", bufs=2, space="PSUM") as pyp, \
             tc.tile_pool(name="st2", bufs=1, space="PSUM") as stp2:
            for dt in range(KD):
                py = pyp.tile([P, SQ], f32, tag="py")
                for j in range(FT // 2):
                    nc.tensor.matmul(py, w2[:, 2 * j:2 * j + 2,
                                             dt * P:(dt + 1) * P],
                                     hsb[:, 2 * j:2 * j + 2, :],
                                     start=(j == 0), stop=(j == FT // 2 - 1),
                                     perf_mode=DR)
                tr = tmp2.tile([P, SQ], f32, tag="tr", bufs=2)
                nc.scalar.activation(out=tr, in_=py, func=ACT.Identity,
                                     bias=aux_ap(AUX_B2, dt), scale=1.0 / WS)
                nc.vector.tensor_tensor(out=r2[:, dt, :], in0=tr,
                                        in1=x1f[:, dt, :], op=ALU.add)
            layer_norm(r2, AUX_G2, AUX_BE2, outsb, None, stp2, tmp2)
            nc.sync.dma_start(out=t_out, in_=outsb)

    nc.compile()
    return nc


def _get_nc():
    if "nc" not in _CACHE:
        _CACHE["nc"] = _build_nc()
    return _CACHE["nc"]


def make_in_maps(q, k, v, Wq, bq, Wk, bk, Wv, bv, Wo, bo, W1, b1, W2, b2,
                 g1, be1, g2, be2):
    f32 = np.float32

    def tile_pd(x, n):  # [n*P] -> [P, n]
        return np.asarray(x, f32).reshape(n, P).T

    def wt8(w, cols):  # torch [out, in] -> [P, in//P, out] fp8, x32
        return np.ascontiguousarray(
            (np.asarray(w, f32).T * WS).reshape(-1, P, cols).transpose(1, 0, 2)
        ).astype(F8)

    aux = np.zeros((P, 48), f32)
    aux[:, AUX_BQ:AUX_BQ + 4] = tile_pd(bq, KD) * ALPHA
    aux[:, AUX_BK:AUX_BK + 4] = tile_pd(bk, KD) * ALPHA
    aux[:, AUX_BO:AUX_BO + 4] = tile_pd(bo, KD)
    aux[:, AUX_B2:AUX_B2 + 4] = tile_pd(b2, KD)
    aux[:, AUX_G1:AUX_G1 + 4] = tile_pd(g1, KD)
    aux[:, AUX_BE1:AUX_BE1 + 4] = tile_pd(be1, KD)
    aux[:, AUX_G2:AUX_G2 + 4] = tile_pd(g2, KD)
    aux[:, AUX_BE2:AUX_BE2 + 4] = tile_pd(be2, KD)
    aux[:, AUX_B1:AUX_B1 + 16] = tile_pd(b1, FT)

    # Wo for DoubleRow: [32, H, 2, KD, P]; within-head feature d = 2p+j
    wodr = np.ascontiguousarray(
        (np.asarray(Wo, f32).T * WS).reshape(H, 32, 2, KD, P)
        .transpose(1, 0, 2, 3, 4)).astype(F8)

    shared = {
        "aux": aux,
        "wq8": wt8(Wq, D), "wk8": wt8(Wk, D), "wv8": wt8(Wv, D),
        "w18": wt8(W1, F), "w28": wt8(W2, D),
        "wodr": wodr,
        "bv32": np.ascontiguousarray(
            np.broadcast_to(np.asarray(bv, f32) * WS, (P, D))).astype(BF16),
    }

    q = np.asarray(q, f32)
    k = np.asarray(k, f32)
    v = np.asarray(v, f32)

    def fm8(x):  # [S, D] -> [P, KD, S] feature-major fp8
        return np.ascontiguousarray(
            x.T.reshape(KD, P, S).transpose(1, 0, 2)).astype(F8)

    kts = [fm8(k[b]) for b in range(B)]
    vts = [fm8(v[b]) for b in range(B)]

    in_maps = []
    for c in range(NCORES):
        b, s0 = c // 4, (c % 4) * SQ
        qt = np.ascontiguousarray(q[b, s0:s0 + SQ, :].T)          # [D, SQ]
        qt4 = np.ascontiguousarray(qt.reshape(KD, P, SQ).transpose(1, 0, 2))
        in_maps.append({
            "qt8": qt4.astype(F8), "qbf": qt4.astype(BF16),
            "kt8": kts[b], "vt8": vts[b], **shared,
        })
    return in_maps


def assemble_out(results):
    out = np.empty((B, S, D), np.float32)
    for c in range(NCORES):
        b, s0 = c // 4, (c % 4) * SQ
        # outT [P, KD, SQ]: feature dt*P+p, token t -> out[t, feature]
        out[b, s0:s0 + SQ, :] = results[c]["outT"].transpose(2, 1, 0).reshape(SQ, D)
    return out


def kernel(**inputs):
    global LAST_RESULT
    import os

    from concourse.bass_utils import run_bass_kernel_spmd

    nc = _get_nc()
    in_maps = make_in_maps(**inputs)
    try:
        res = run_bass_kernel_spmd(nc, in_maps, core_ids=list(range(NCORES)))
    except ModuleNotFoundError:
        # BASS_TRACE set but this container has no axon NTFF profile hook
        # (antenv.axon_hooks missing) — rerun untraced.
        os.environ["BASS_NEVER_TRACE"] = "1"
        res = run_bass_kernel_spmd(nc, in_maps, core_ids=list(range(NCORES)))
    LAST_RESULT = res
    return assemble_out(res.results)
